# revision 28
# baseline (speedup 1.0000x reference)
"""Grouped-Query Attention (B=2, S=2048, D=2048, 16 Q heads / 4 KV heads,
hd=128, RoPE, causal) on 8 trn2 NeuronCores.

Sharding: mesh = 2 (batch) x 4 (KV-head groups).  Core c = b*4 + g gets
batch b and KV head g together with its 4 query heads (tensor parallel on
the head dim: q/k/v projection output dim and o-proj input dim).  Each core
produces a partial y[b] (o-proj over its 512 input dims); host sums the 4
partials per batch.

v2: all matmuls in bf16 (f32 PSUM accumulation), softmax denominator
accumulated on GpSimd (instead of per-block PE matmuls), fast approximate
reciprocal, causal-diagonal trimming, and a software-pipelined global
schedule interleaving projection chunks / score+exp passes / av+o-proj
passes so the tensor engine never idles long enough for the HAM clock gate
to re-throttle it to 1.2 GHz.
"""

import os

import numpy as np
import ml_dtypes

S = 2048
D = 2048
HD = 128
NQH = 16
NKVH = 4
GROUPS = NQH // NKVH  # 4 q heads per kv head
O = GROUPS * HD  # 512 per-core q/o slice
NB = 2
NCORES = 8
SCALE = 1.0 / float(np.sqrt(np.float32(HD)))
NEG = -1.0e30

SBLK = 512  # seq block for projections / sq block in attention
NKB = S // HD  # 16 128-blocks along seq
NSB = S // SBLK  # 4 512-blocks along seq
NDB = D // HD  # 16 d blocks

BF16 = ml_dtypes.bfloat16

LAST_EXEC_NS = None
LAST_TRACE = None

_CACHE = {}


def _rope_tables():
    k = np.arange(0, HD, 2)[: HD // 2].astype(np.float32)
    inv_freq = (1.0 / 10000.0 ** (k / HD)).astype(np.float32)
    positions = np.arange(S, dtype=np.float32)
    ang = positions[:, None] * inv_freq[None, :]  # [S, 64]
    ang = np.concatenate([ang, ang], axis=-1)  # [S, 128]
    cosT = np.cos(ang).astype(BF16).T  # [128, S]
    sinT = np.sin(ang).astype(BF16).T
    return np.ascontiguousarray(cosT), np.ascontiguousarray(sinT)


def _mask_table():
    # triangular mask for the first 128 columns of any trimmed diagonal
    # block: mask[i, c] = 0 if i <= c else NEG (identical for every block)
    i = np.arange(HD)[:, None]
    c = np.arange(HD)[None, :]
    return np.where(i <= c, 0.0, NEG).astype(BF16)


def _shift_table():
    # rot = P @ q  with rot[i] = -q[i+64] (i<64), q[i-64] (i>=64); ship P.T
    P = np.zeros((HD, HD), dtype=np.float32)
    h = HD // 2
    P[np.arange(h), np.arange(h) + h] = -1.0
    P[np.arange(h) + h, np.arange(h)] = 1.0
    return np.ascontiguousarray(P.T.astype(BF16))


def _build_program():
    import concourse.bass as bass
    import concourse.mybir as mybir
    from concourse.tile import TileContext

    f32 = mybir.dt.float32
    f32r = mybir.dt.float32r
    bf16 = mybir.dt.bfloat16
    EXP = mybir.ActivationFunctionType.Exp

    nc = bass.Bass()

    xT = nc.declare_dram_parameter("xT", [D, S], bf16, isOutput=False)
    wqP = nc.declare_dram_parameter("wqP", [128, NDB * O], bf16, isOutput=False)
    wkP = nc.declare_dram_parameter("wkP", [128, NDB * HD], bf16, isOutput=False)
    wvP = nc.declare_dram_parameter("wvP", [128, NDB * HD], bf16, isOutput=False)
    woP = nc.declare_dram_parameter("woP", [128, GROUPS * D], bf16, isOutput=False)
    cosT = nc.declare_dram_parameter("cosT", [HD, S], bf16, isOutput=False)
    sinT = nc.declare_dram_parameter("sinT", [HD, S], bf16, isOutput=False)
    maskT = nc.declare_dram_parameter("maskT", [HD, HD], bf16, isOutput=False)
    shiftPT = nc.declare_dram_parameter("shiftPT", [HD, HD], bf16, isOutput=False)
    ident = nc.declare_dram_parameter("ident", [HD, HD], bf16, isOutput=False)
    onescol = nc.declare_dram_parameter("onescol", [HD, 1], f32r, isOutput=False)
    onesrow = nc.declare_dram_parameter("onesrow", [1, HD], f32r, isOutput=False)
    y = nc.declare_dram_parameter("y", [S, D], bf16, isOutput=True)

    with TileContext(nc) as tc:
        with (
            tc.tile_pool(name="persist", bufs=1) as pp,
            tc.tile_pool(name="ps", bufs=8, space="PSUM") as ps,
            tc.tile_pool(name="xts", bufs=16) as xpool,
            tc.tile_pool(name="raws", bufs=3) as rawpool,
            tc.tile_pool(name="tmps", bufs=3) as tmppool,
            tc.tile_pool(name="es", bufs=76) as epool,
            tc.tile_pool(name="eaccs", bufs=6) as eaccpool,
            tc.tile_pool(name="denrs", bufs=2) as denrpool,
            tc.tile_pool(name="bcss", bufs=2) as bcspool,
            tc.tile_pool(name="aos", bufs=2) as aopool,
            tc.tile_pool(name="ysbs", bufs=2) as ypool,
        ):
            # weight slices share the e-tile pool: they occupy 1KB slots only
            # until P3's matmuls consume them, long before the e-tile peak
            wq_t = [epool.tile([128, O], bf16, name=f"wq{db}", tag="e") for db in range(NDB)]
            wk_t = [epool.tile([128, HD], bf16, name=f"wk{db}", tag="e") for db in range(NDB)]
            wv_t = [epool.tile([128, HD], bf16, name=f"wv{db}", tag="e") for db in range(NDB)]
            wo_sb = pp.tile([128, GROUPS * D], bf16, name="wo_sb")
            cos_sb = pp.tile([HD, S], bf16, name="cos_sb")
            sin_sb = pp.tile([HD, S], bf16, name="sin_sb")
            mask_sb = pp.tile([HD, HD], bf16, name="mask_sb")
            shift_sb = pp.tile([HD, HD], bf16, name="shift_sb")
            id_sb = pp.tile([HD, HD], bf16, name="id_sb")
            ones_sb = pp.tile([HD, 1], f32r, name="ones_sb")
            oner_sb = pp.tile([1, HD], f32r, name="oner_sb")
            q_sb = pp.tile([128, GROUPS * S], bf16, name="q_sb")  # per head [hd, S]
            k_sb = pp.tile([128, S], bf16, name="k_sb")
            v_sb = pp.tile([128, NKB * HD], bf16, name="v_sb")

            nc.sync.dma_start(out=cos_sb[:], in_=cosT[:])
            nc.sync.dma_start(out=sin_sb[:], in_=sinT[:])
            nc.sync.dma_start(out=mask_sb[:], in_=maskT[:])
            nc.sync.dma_start(out=shift_sb[:], in_=shiftPT[:])
            nc.sync.dma_start(out=id_sb[:], in_=ident[:])
            nc.sync.dma_start(out=ones_sb[:], in_=onescol[:])
            nc.sync.dma_start(out=oner_sb[:], in_=onesrow[:])

            accs = {}
            e_t = {}
            eacc_t = {}
            den4_t = {}
            denr_t = {}
            av_t = {}
            ao_t = {}

            xts = {}

            def xt_dma(pair, db):
                # double-width x tile [128, 1024] covering two P segments,
                # alternated across both HWDGE queues (SP + Activation) so
                # delivery keeps pace with the 1.28us/chunk PE consumption
                xt = xpool.tile([128, 2 * SBLK], bf16, name=f"xt{pair}_{db}", tag="xt")
                eng = nc.sync if db % 2 == 0 else nc.scalar
                eng.dma_start(
                    out=xt[:],
                    in_=xT[db * 128 : (db + 1) * 128, pair * 1024 : (pair + 1) * 1024],
                )
                xts[db] = xt

            def proj_chunk(sb, db):
                if db == 0:
                    accs[sb] = [
                        ps.tile([128, SBLK], f32, name=f"acc{sb}_{i}", tag="ps")
                        for i in range(6)
                    ]
                acc = accs[sb]
                if sb == 0:
                    nc.sync.dma_start(
                        out=wq_t[db][:], in_=wqP[:, db * O : (db + 1) * O]
                    )
                    nc.sync.dma_start(
                        out=wk_t[db][:], in_=wkP[:, db * HD : (db + 1) * HD]
                    )
                    nc.sync.dma_start(
                        out=wv_t[db][:], in_=wvP[:, db * HD : (db + 1) * HD]
                    )
                if sb == 0:
                    xt_dma(0, db)
                half = sb % 2
                xt = xts[db][:, half * SBLK : (half + 1) * SBLK]
                st = db == 0
                sp = db == NDB - 1
                for ob in range(GROUPS):
                    nc.tensor.matmul(
                        acc[ob][:],
                        wq_t[db][:, ob * HD : (ob + 1) * HD],
                        xt,
                        start=st,
                        stop=sp,
                    )
                nc.tensor.matmul(acc[4][:], wk_t[db][:], xt, start=st, stop=sp)
                nc.tensor.matmul(acc[5][:], wv_t[db][:], xt, start=st, stop=sp)

            def rope(sb):
                acc = accs[sb]
                sl = slice(sb * SBLK, (sb + 1) * SBLK)
                for i in range(5):
                    raw = rawpool.tile([128, SBLK], bf16, name=f"raw{sb}_{i}", tag="raw")
                    nc.scalar.copy(raw[:], acc[i][:])
                    rot = ps.tile([128, SBLK], f32, name=f"rot{sb}_{i}", tag="ps")
                    nc.tensor.matmul(
                        rot[:], shift_sb[:], raw[:], start=True, stop=True
                    )
                    t1 = tmppool.tile([128, SBLK], bf16, name=f"t1_{sb}_{i}", tag="tmp")
                    nc.vector.tensor_mul(t1[:], raw[:], cos_sb[:, sl])
                    t2 = tmppool.tile([128, SBLK], bf16, name=f"t2_{sb}_{i}", tag="tmp")
                    nc.vector.tensor_mul(t2[:], rot[:], sin_sb[:, sl])
                    dst = (
                        q_sb[:, i * S + sb * SBLK : i * S + (sb + 1) * SBLK]
                        if i < 4
                        else k_sb[:, sl]
                    )
                    nc.vector.tensor_add(dst, t1[:], t2[:])

            def vT(sb):
                vst = rawpool.tile([128, SBLK], bf16, name=f"vst{sb}", tag="raw")
                nc.scalar.copy(vst[:], accs[sb][5][:])
                for sub in range(SBLK // HD):
                    vt = ps.tile([128, HD], bf16, name=f"vt{sb}_{sub}", tag="ps")
                    nc.tensor.transpose(
                        vt[:], vst[:, sub * HD : (sub + 1) * HD], id_sb[:]
                    )
                    kb = sb * 4 + sub
                    nc.vector.tensor_copy(v_sb[:, kb * HD : (kb + 1) * HD], vt[:])
                del accs[sb]

            def A_unit(sq, h):
                nsk = 4 * sq + 4
                eacc = eaccpool.tile(
                    [128, SBLK], f32r, name=f"eacc{sq}_{h}", tag="eacc"
                )
                eacc_t[(sq, h)] = eacc
                for kb in range(nsk):
                    j = kb - 4 * sq
                    s0 = 128 * j if j > 0 else 0
                    w = SBLK - s0
                    qoff = h * S + sq * SBLK + s0
                    sc = ps.tile([128, w], f32, name=f"sc{sq}_{h}_{kb}", tag="ps")
                    nc.tensor.matmul(
                        sc[:],
                        k_sb[:, kb * HD : (kb + 1) * HD],
                        q_sb[:, qoff : qoff + w],
                        start=True,
                        stop=True,
                    )
                    if j >= 0:
                        nc.vector.tensor_add(sc[:, 0:HD], sc[:, 0:HD], mask_sb[:])
                    e = epool.tile([128, w], bf16, name=f"e{sq}_{h}_{kb}", tag="e")
                    nc.scalar.activation(e[:], sc[:], EXP, scale=SCALE)
                    e_t[(sq, h, kb)] = e
                    with nc.allow_low_precision(reason="softmax denom accum f32r"):
                        if kb == 0:
                            nc.vector.tensor_copy(eacc[:], e[:])
                        else:
                            nc.gpsimd.tensor_add(
                                eacc[:, s0:SBLK], eacc[:, s0:SBLK], e[:]
                            )

            def B_av(sq, h):
                nsk = 4 * sq + 4
                av = ps.tile([128, SBLK], f32, name=f"av{sq}_{h}", tag="ps")
                av_t[(sq, h)] = av
                for kb in range(nsk):
                    j = kb - 4 * sq
                    s0 = 128 * j if j > 0 else 0
                    nc.tensor.matmul(
                        av[:, s0:SBLK],
                        v_sb[:, kb * HD : (kb + 1) * HD],
                        e_t[(sq, h, kb)][:],
                        start=(kb == 0),
                        stop=(kb == nsk - 1),
                    )

            def B_den(sq, h):
                den = ps.tile([1, SBLK], f32, name=f"den{sq}_{h}", tag="ps")
                nc.tensor.matmul(
                    den[:],
                    ones_sb[:],
                    eacc_t[(sq, h)][:],
                    start=True,
                    stop=True,
                )
                denr = denrpool.tile([1, SBLK], f32, name=f"denr{sq}_{h}", tag="denr")
                nc.vector.reciprocal_approx_fast(out=denr[:], in_=den[:])
                denrr = denrpool.tile(
                    [1, SBLK], f32r, name=f"denrr{sq}_{h}", tag="denrr"
                )
                denr_t[(sq, h)] = denrr
                with nc.allow_low_precision(reason="f32r rounding of softmax denom"):
                    nc.vector.tensor_copy(denrr[:], denr[:])

            def B_bc(sq, h):
                if h == 0:
                    ao_t[sq] = aopool.tile(
                        [128, GROUPS * SBLK], bf16, name=f"ao{sq}", tag="ao"
                    )
                bc = ps.tile([128, SBLK], f32, name=f"bc{sq}_{h}", tag="ps")
                nc.tensor.matmul(
                    bc[:],
                    oner_sb[:],
                    denr_t[(sq, h)][:],
                    start=True,
                    stop=True,
                )
                bcs = bcspool.tile([128, SBLK], f32, name=f"bcs{sq}_{h}", tag="bcs")
                nc.vector.tensor_copy(bcs[:], bc[:])
                nc.vector.tensor_mul(
                    ao_t[sq][:, h * SBLK : (h + 1) * SBLK], av_t[(sq, h)][:], bcs[:]
                )

            def O_unit(sq, sub):
                ysb = ypool.tile([128, D], bf16, name=f"ysb{sq}_{sub}", tag="ysb")
                for dc in range(D // SBLK):
                    yt = ps.tile([128, SBLK], f32, name=f"yt{sq}_{sub}_{dc}", tag="ps")
                    for ob in range(GROUPS):
                        nc.tensor.matmul(
                            yt[:],
                            ao_t[sq][
                                :, ob * SBLK + sub * HD : ob * SBLK + (sub + 1) * HD
                            ],
                            wo_sb[:, ob * D + dc * SBLK : ob * D + (dc + 1) * SBLK],
                            start=(ob == 0),
                            stop=(ob == GROUPS - 1),
                        )
                    nc.vector.tensor_copy(ysb[:, dc * SBLK : (dc + 1) * SBLK], yt[:])
                nc.sync.dma_start(
                    out=y[sq * SBLK + sub * HD : sq * SBLK + (sub + 1) * HD, :],
                    in_=ysb[:],
                )

            # ---------------- global software-pipelined schedule
            def P_seg(sb):
                for db in range(NDB):
                    proj_chunk(sb, db)
                rope(sb)
                vT(sb)

            # Schedule rationale: ACT is a serial exp engine (~0.9us per
            # [128,512] tile); each B(sq) needs all of A(sq)'s exps done, so
            # A(sq) emission must lead B(sq) by enough PE work to cover the
            # exp stream.  O(0)/O(1)/O(2) are pure outputs with no
            # consumers, so they serve as late PE filler opposite the sq2/3
            # exp backlog.
            P_seg(0)
            nc.sync.dma_start(out=wo_sb[:], in_=woP[:])
            P_seg(1)
            # prefetch the second x half now: slots free as P1 consumes the
            # first-generation tiles, and the ACT-queue dma issues land
            # before the exp stream floods that queue
            for db in range(NDB):
                xt_dma(1, db)
            for h in range(4):
                A_unit(0, h)
            P_seg(2)
            B_av(0, 0)
            B_av(0, 1); B_den(0, 0)
            B_av(0, 2); B_den(0, 1); B_bc(0, 0)
            B_av(0, 3); B_den(0, 2); B_bc(0, 1)
            B_den(0, 3); B_bc(0, 2)
            B_bc(0, 3)
            for h in range(4):
                A_unit(1, h)
            P_seg(3)
            A_unit(2, 0)
            A_unit(2, 1)
            B_av(1, 0)
            B_av(1, 1); B_den(1, 0)
            B_av(1, 2); B_den(1, 1); B_bc(1, 0)
            B_av(1, 3); B_den(1, 2); B_bc(1, 1)
            B_den(1, 3); B_bc(1, 2)
            B_bc(1, 3)
            A_unit(2, 2)
            A_unit(2, 3)
            for sub in range(4):
                O_unit(0, sub)
            A_unit(3, 0)
            B_av(2, 0)
            A_unit(3, 1)
            B_av(2, 1); B_den(2, 0)
            A_unit(3, 2)
            B_av(2, 2); B_den(2, 1); B_bc(2, 0)
            A_unit(3, 3)
            B_av(2, 3); B_den(2, 2); B_bc(2, 1)
            B_den(2, 3); B_bc(2, 2)
            B_bc(2, 3)
            for sub in range(4):
                O_unit(1, sub)
            B_av(3, 0)
            O_unit(2, 0); O_unit(2, 1)
            B_av(3, 1); B_den(3, 0)
            O_unit(2, 2); O_unit(2, 3)
            B_av(3, 2); B_den(3, 1); B_bc(3, 0)
            B_av(3, 3); B_den(3, 2); B_bc(3, 1)
            B_den(3, 3); B_bc(3, 2)
            B_bc(3, 3)
            for sub in range(4):
                O_unit(3, sub)

    # populate .instr bytes for extended-inst InstISA subclasses (the
    # custom-DVE reciprocal) — raw Bass doesn't run this pass and the NEFF
    # compiler errors with "ISA wrong length" without it
    mybir.codegen_inst_isa_subclasses(nc)
    _split_matmul_waits(nc, mybir)
    return nc


def _split_matmul_waits(nc, mybir):
    """TRN2 instructions can carry only one HW sync-wait command; Tile
    sometimes attaches several.  Move the extras onto nofuse nops on the
    same engine inserted just before the instruction."""
    for f in nc.m.functions:
        for bb in f.blocks:
            insts = bb.instructions
            fixes = []
            for idx, inst in enumerate(insts):
                si = inst.sync_info
                if si is None or len(si.on_wait) <= 1:
                    continue
                fixes.append((idx, inst, list(si.on_wait), list(si.on_update)))
            for idx, inst, waits, updates in reversed(fixes):
                inst.sync_info = mybir.SyncInfo(on_wait=[waits[-1]], on_update=updates)
                for w in reversed(waits[:-1]):
                    nop = mybir.InstNoOp(
                        name=nc.get_next_instruction_name(), ins=[], outs=[]
                    )
                    nop.engine = inst.engine
                    nop.bass_nofuse = True
                    nop.sync_info = mybir.SyncInfo(on_wait=[w], on_update=[])
                    insts.insert(idx, nop)


def _per_core_inputs(x, Wq, Wk, Wv, Wo):
    cosT, sinT = _rope_tables()
    maskT = _mask_table()
    shiftPT = _shift_table()
    ident = np.eye(HD, dtype=np.float32).astype(BF16)
    onescol = np.ones((HD, 1), dtype=np.float32)
    onesrow = np.ones((1, HD), dtype=np.float32)
    in_maps = []
    for b in range(NB):
        xTb = np.ascontiguousarray(x[b].T).astype(BF16)
        for g in range(NKVH):
            wqT = Wq[g * O : (g + 1) * O, :].T  # [D, O]
            wkT = Wk[g * HD : (g + 1) * HD, :].T
            wvT = Wv[g * HD : (g + 1) * HD, :].T
            woT = Wo[:, g * O : (g + 1) * O].T  # [O, D]
            in_maps.append(
                {
                    "xT": xTb,
                    "wqP": np.ascontiguousarray(
                        wqT.reshape(NDB, 128, O).transpose(1, 0, 2).reshape(128, NDB * O)
                    ).astype(BF16),
                    "wkP": np.ascontiguousarray(
                        wkT.reshape(NDB, 128, HD).transpose(1, 0, 2).reshape(128, NDB * HD)
                    ).astype(BF16),
                    "wvP": np.ascontiguousarray(
                        wvT.reshape(NDB, 128, HD).transpose(1, 0, 2).reshape(128, NDB * HD)
                    ).astype(BF16),
                    "woP": np.ascontiguousarray(
                        woT.reshape(GROUPS, 128, D).transpose(1, 0, 2).reshape(128, GROUPS * D)
                    ).astype(BF16),
                    "cosT": cosT,
                    "sinT": sinT,
                    "maskT": maskT,
                    "shiftPT": shiftPT,
                    "ident": ident,
                    "onescol": onescol,
                    "onesrow": onesrow,
                }
            )
    return in_maps


def kernel(x, Wq, Wk, Wv, Wo):
    global LAST_EXEC_NS, LAST_TRACE
    from concourse.bass_utils import run_bass_kernel_spmd

    if "nc" not in _CACHE:
        _CACHE["nc"] = _build_program()
    nc = _CACHE["nc"]

    x = np.asarray(x)
    in_maps = _per_core_inputs(
        x, np.asarray(Wq), np.asarray(Wk), np.asarray(Wv), np.asarray(Wo)
    )
    trace = bool(os.environ.get("KERNEL_PROFILE"))
    res = run_bass_kernel_spmd(
        nc, in_maps, core_ids=list(range(NCORES)), trace=trace
    )
    globals()["LAST_RESULT"] = res
    LAST_EXEC_NS = res.exec_time_ns
    LAST_TRACE = getattr(res, "profile_json", None)
    out = np.empty((NB, S, D), dtype=np.float32)
    for b in range(NB):
        acc = res.results[b * NKVH]["y"].astype(np.float32)
        for g in range(1, NKVH):
            acc += res.results[b * NKVH + g]["y"].astype(np.float32)
        out[b] = acc
    return out


if __name__ == "__main__":
    # quick build-only syntax/assert check (no device)
    nc = _build_program()
    n_inst = sum(len(bb.instructions) for f in nc.m.functions for bb in f.blocks)
    print(f"build OK: {n_inst} instructions")
